# revision 14
# baseline (speedup 1.0000x reference)
"""Bahdanau-attention kernel for Trainium2 (8 NeuronCores, Bass/Tile).

Computation (reference, fp32):
    Wh  = hidden @ W_w.T + W_b                      # [B, H]
    Ue  = einsum('bse,he->bsh', enc^T, U_w) + U_b   # [B, S, H]
    en  = tanh(Wh[:,None,:] + Ue) @ v_w[0]          # [B, S]
    out = softmax(where(mask, -1e10, en), axis=1)

Strategy
- Data-parallel over batch: 8 batches per core, weights replicated.
- Masked positions contribute exactly 0 to the softmax (exp(-1e10) = 0
  in fp32), so the host packs only the unmasked s-columns per batch row
  and scatters results back; the device computes energies only for
  packed columns. This is exact, not an approximation.
- Sorted-slot packing: the 64 rows are sorted by unmasked count and
  rank-grouped into 8 slots (one row per core per slot), so each slot's
  padded width is the max of 8 *similar* counts instead of the global
  max. Fully-masked rows are uniform 1/S by definition, fixed on host.
- Main matmul out[h, s] = U_w.T-chunk (stationary) x enc-chunk (moving)
  in bf16 with fp32 PSUM accumulation; 16 k-chunks of 128 accumulate in
  one PSUM bank per (slot, h-chunk).
- All DMA is laid out host-side to be per-partition contiguous so each
  logical transfer is a single large descriptor stream (fast ramp).
- A short burst of dummy matmuls on scratch SBUF warms the PE HAM clock
  gate (1.2 -> 2.4 GHz) during the initial DMA ramp, so real matmuls
  run at full clock from the start.
- Wh + W_b + U_b is folded into the tanh as a per-partition ACT bias;
  the Wh chains interleave with slot 0's main blocks so PE work paces
  the startup DMA stream.
- The v-projection runs on the (otherwise idle) Vector engine: each
  tanh tile is scaled by its per-partition v chunk and accumulated over
  h-chunks in SBUF; a single ones-vector M=1 matmul per slot does the
  final partition reduce.  The LAST slot instead reduces via direct
  lhsT=v-chunk matmuls accumulating in PSUM, which removes the vector
  chain from the kernel's tail critical path.
- The device applies exp per packed row; the softmax normalization
  (sum + divide over each row's unmasked entries) happens in the host
  unpack loop, removing the reciprocal/broadcast chain from the tail.
"""

import numpy as np
import ml_dtypes

B, S, H, E = 64, 512, 1024, 2048
NCORES = 8
BL = B // NCORES          # batches (slots) per core
HC = H // 128             # h chunks
EC = E // 128             # e (contraction) chunks
KC = H // 128             # k chunks for the Wh matmul

bf16 = ml_dtypes.bfloat16

_CACHE = {}

N_WARM = 16               # dummy warm-up matmuls (N=256) during DMA ramp
F_WH = 4                  # dummy filler (N=128) between Wh chains
F_MAIN = 22               # dummy filler (N=128) before first main chain


def _build_nc(Ws):
    """Per-core program; Ws = tuple of 8 slot widths (mult of 4, <=512),
    in processing order (descending)."""
    import concourse.mybir as mybir
    import concourse.tile as tile
    from concourse import bacc

    F32 = mybir.dt.float32
    BF = mybir.dt.bfloat16
    AF = mybir.ActivationFunctionType

    Ws = list(Ws)
    TOT = sum(Ws)
    off = [0]
    for w in Ws:
        off.append(off[-1] + w)

    nc = bacc.Bacc(num_swdge_queues=4)
    # enc packed: [p, b, ec, s]  (per-partition contiguous per slot)
    enc_t = nc.declare_dram_parameter("enc_t", [128, EC * TOT], BF, isOutput=False)
    # U_w.T pre-chunked: [p(=e%128), hc, ec, v(=h%128)]
    uwT = nc.declare_dram_parameter("uwT", [128, HC * EC * 128], BF, isOutput=False)
    # W_w.T re-chunked: [p(=k%128), hc, kc, v(=h%128)]
    wwT = nc.declare_dram_parameter("wwT", [128, HC * KC * 128], BF, isOutput=False)
    # misc = hidT [.,0:KC*BL] | vt [.,KC*BL:+HC] | bc [.,KC*BL+HC:+HC]
    MISC = KC * BL + 2 * HC
    misc = nc.declare_dram_parameter("misc", [128, MISC], BF, isOutput=False)
    out_d = nc.declare_dram_parameter("out", [1, TOT], F32, isOutput=True)

    def enc_base(b, ec):
        return EC * off[b] + ec * Ws[b]

    with tile.TileContext(nc) as tc:
        with (
            tc.tile_pool(name="const", bufs=1) as cst,
            tc.tile_pool(name="wpool", bufs=1) as wp,
            tc.tile_pool(name="thp", bufs=6) as thp,
            tc.tile_pool(name="accp", bufs=3) as accp,
            tc.tile_pool(name="pup", bufs=4, space="PSUM") as pup,
            tc.tile_pool(name="pep", bufs=2, space="PSUM") as pep,
            tc.tile_pool(name="pwp", bufs=1, space="PSUM") as pwp,
            tc.tile_pool(name="pwarm", bufs=1, space="PSUM") as pwarm,
        ):
            # ---- PE warm-up on scratch data (HAM clock-gate release) ---
            warm_sb = cst.tile([128, 384], BF, tag="warm")
            nc.vector.memset(warm_sb[:], 1.0)
            warm_ps = pwarm.tile([128, 256], F32, tag="wps")
            for _ in range(N_WARM):
                nc.tensor.matmul(
                    warm_ps[:], lhsT=warm_sb[:, 0:128], rhs=warm_sb[:, 128:384],
                    start=True, stop=True,
                )

            def filler(n):
                # N=128 dummy matmuls: keep the PE (and its HAM activity
                # window) busy while waiting for DMA, in fine increments
                for _ in range(n):
                    nc.tensor.matmul(
                        warm_ps[:, 0:128], lhsT=warm_sb[:, 0:128],
                        rhs=warm_sb[:, 128:256], start=True, stop=True,
                    )

            # ---- constants / weights ------------------------------------
            misc_sb = cst.tile([128, MISC], BF, tag="misc")
            VT0 = KC * BL          # vt col offset in misc
            BC0 = KC * BL + HC     # bc col offset in misc
            ww_sb = wp.tile([128, HC * KC * 128], BF, tag="ww")
            uw_sb = wp.tile([128, HC * EC * 128], BF, tag="uw")
            enc_sb = wp.tile([128, EC * TOT], BF, tag="enc")

            WWC = KC * 128   # ww cols per hc
            UWC = EC * 128   # uw cols per hc

            def ww_sl(hc):
                return slice(hc * WWC, (hc + 1) * WWC)

            # q1 = sync, q2 = gpsimd.  Order = PE need order.  The Wh
            # prologue (all 8 chains, run before the main loop) needs
            # misc+ww only, so those go first and pace the PE while the
            # bigger uw/enc stream lands behind them.  ww even chunks on
            # q1, odd on q2, so arrival order matches Wh chain order.
            nc.gpsimd.dma_start(misc_sb[:], misc[:])
            for hc in range(0, HC, 2):
                nc.sync.dma_start(ww_sb[:, ww_sl(hc)], wwT[:, ww_sl(hc)])
            for hc in range(1, HC, 2):
                nc.gpsimd.dma_start(ww_sb[:, ww_sl(hc)], wwT[:, ww_sl(hc)])
            UH = UWC // 2
            EG = EC // 4
            for b01 in range(2):
                a = b01 * UWC
                nc.sync.dma_start(uw_sb[:, a:a + UH], uwT[:, a:a + UH])
                nc.gpsimd.dma_start(
                    uw_sb[:, a + UH:a + UWC], uwT[:, a + UH:a + UWC])
                # slot-0/1 enc in 4 ec-groups, alternating queues (fine
                # grain so the first chains can start on partial data)
                for g in range(4):
                    s0 = enc_base(b01, g * EG)
                    bnd = (enc_base(b01, (g + 1) * EG)
                           if g < 3 else enc_base(b01 + 1, 0))
                    eng = (nc.sync, nc.gpsimd)[g % 2]
                    eng.dma_start(enc_sb[:, s0:bnd], enc_t[:, s0:bnd])
            # remaining uw chunks, then remaining enc slots
            for hc in range(2, HC):
                a = hc * UWC
                eng = (nc.sync, nc.gpsimd)[hc % 2]
                eng2 = (nc.gpsimd, nc.sync)[hc % 2]
                eng.dma_start(uw_sb[:, a:a + UH], uwT[:, a:a + UH])
                eng2.dma_start(uw_sb[:, a + UH:a + UWC], uwT[:, a + UH:a + UWC])
            for b in range(2, BL):
                a = enc_base(b, 0)
                m = enc_base(b, EC // 2)
                bnd = enc_base(b + 1, 0) if b + 1 < BL else EC * TOT
                eng = (nc.sync, nc.gpsimd)[b % 2]
                eng2 = (nc.gpsimd, nc.sync)[b % 2]
                eng.dma_start(enc_sb[:, a:m], enc_t[:, a:m])
                eng2.dma_start(enc_sb[:, m:bnd], enc_t[:, m:bnd])

            bias_sb = cst.tile([128, HC * BL], F32, tag="bias")
            res_sb = cst.tile([1, TOT], F32, tag="res")
            ones_sb = cst.tile([128, 1], BF, tag="ones")
            nc.vector.memset(ones_sb[:], 1.0)

            # ---- Wh prologue: all 8 chains upfront ----------------------
            # These need only hid+ww (~2.3 MB, first in DMA order), so
            # they keep the PE warm while the uw/enc stream lands.
            for hc in range(HC):
                pw = pwp.tile([128, BL], F32, tag="pw")
                for kc in range(KC):
                    nc.tensor.matmul(
                        pw[:],
                        lhsT=ww_sb[:, hc * WWC + kc * 128:
                                   hc * WWC + (kc + 1) * 128],
                        rhs=misc_sb[:, kc * BL:(kc + 1) * BL],
                        start=(kc == 0),
                        stop=(kc == KC - 1),
                    )
                nc.vector.tensor_tensor(
                    bias_sb[:, hc * BL:(hc + 1) * BL], pw[:],
                    misc_sb[:, BC0 + hc:BC0 + hc + 1].to_broadcast([128, BL]),
                    mybir.AluOpType.add,
                )
                if hc < HC - 1:
                    filler(F_WH)

            filler(F_MAIN)

            # ---- main loop over slots -----------------------------------
            for b in range(BL):
                W = Ws[b]
                last = b == BL - 1
                pe_ = pep.tile([1, W], F32, tag="pe")
                if not last:
                    acc = accp.tile([128, W], F32, tag="acc")
                for hc in range(HC):
                    pu = pup.tile([128, W], F32, tag="pu")
                    for ec in range(EC):
                        a = enc_base(b, ec)
                        nc.tensor.matmul(
                            pu[:],
                            lhsT=uw_sb[:, hc * UWC + ec * 128:
                                       hc * UWC + (ec + 1) * 128],
                            rhs=enc_sb[:, a:a + W],
                            start=(ec == 0),
                            stop=(ec == EC - 1),
                        )
                    th = thp.tile([128, W], BF, tag="th")
                    nc.scalar.activation(
                        th[:], pu[:], AF.Tanh,
                        bias=bias_sb[:, hc * BL + b:hc * BL + b + 1],
                    )
                    if last:
                        # direct v-chunk reduce on PE: shortest tail path
                        nc.tensor.matmul(
                            pe_[0:1, :], lhsT=misc_sb[:, VT0 + hc:VT0 + hc + 1],
                            rhs=th[:],
                            start=(hc == 0), stop=(hc == HC - 1),
                        )
                    else:
                        # v-weighting on the (otherwise idle) Vector engine
                        vcol = misc_sb[:, VT0 + hc:VT0 + hc + 1].to_broadcast([128, W])
                        if hc == 0:
                            nc.vector.tensor_tensor(
                                acc[:], th[:], vcol, mybir.AluOpType.mult)
                        else:
                            tmp = thp.tile([128, W], F32, tag="tmp")
                            nc.vector.tensor_tensor(
                                tmp[:], th[:], vcol, mybir.AluOpType.mult)
                            nc.vector.tensor_add(acc[:], acc[:], tmp[:])
                if not last:
                    # single partition-reduce matmul replaces the 8 v-dots
                    accb = thp.tile([128, W], BF, tag="accb")
                    nc.vector.tensor_copy(accb[:], acc[:])
                    nc.tensor.matmul(
                        pe_[0:1, :], lhsT=ones_sb[:, 0:1], rhs=accb[:],
                        start=True, stop=True,
                    )

                # ---- exp over packed columns; normalization on host -----
                sl = slice(off[b], off[b] + W)
                nc.scalar.activation(res_sb[0:1, sl], pe_[0:1, :], AF.Exp)
                nc.sync.dma_start(out_d[0:1, sl], res_sb[0:1, sl])

    nc.finalize()
    return nc


def _prep_inputs(hidden, encoder_outputs, mask, W_w, W_b, U_w, U_b, v_w):
    enc_bf = encoder_outputs.astype(bf16)          # [S, B, E]
    # U_w.T [E, H] -> [p(=e%128), hc, ec, v(=h%128)]
    uwT_np = np.ascontiguousarray(U_w.T).astype(bf16)
    uwT_np = np.ascontiguousarray(
        uwT_np.reshape(EC, 128, HC, 128).transpose(1, 2, 0, 3)
    ).reshape(128, HC * EC * 128)
    wwT_np = np.ascontiguousarray(W_w.T).astype(bf16)
    wwT_np = np.ascontiguousarray(
        wwT_np.reshape(KC, 128, HC, 128).transpose(1, 2, 0, 3)
    ).reshape(128, HC * KC * 128)
    vt_np = np.ascontiguousarray(v_w[0].reshape(HC, 128).T).astype(bf16)
    bc_np = np.ascontiguousarray((W_b + U_b).reshape(HC, 128).T).astype(bf16)

    idx_all = [np.nonzero(~mask[i])[0] for i in range(B)]
    counts = np.array([len(ix) for ix in idx_all])

    # sorted-slot packing: rank-group rows into 8 slots of 8 (one per core)
    order = np.argsort(-counts, kind="stable")
    rows = order.reshape(BL, NCORES)       # rows[b][c] = global row index
    Ws = tuple(int(max(4, 4 * -(-counts[rows[b]].max() // 4)))
               for b in range(BL))
    TOT = sum(Ws)
    off = np.concatenate([[0], np.cumsum(Ws)]).astype(int)

    in_maps = []
    for c in range(NCORES):
        crows = rows[:, c]                                       # slot -> row
        # enc for this core's rows: [E, BL, S]
        enc_c = np.ascontiguousarray(enc_bf[:, crows, :].transpose(2, 1, 0))
        enc_p = np.zeros((128, EC * TOT), bf16)
        for b in range(BL):
            ix = idx_all[crows[b]]
            cnt = len(ix)
            if cnt:
                # [E, cnt] -> [EC, 128, cnt] -> [128, EC, cnt]
                g = enc_c[:, b, ix].reshape(EC, 128, cnt).transpose(1, 0, 2)
                v = enc_p[:, EC * off[b]:EC * off[b + 1]].reshape(128, EC, Ws[b])
                v[:, :, :cnt] = g
        hid_c = hidden[crows].astype(bf16)                       # [BL, H]
        hidT_c = np.ascontiguousarray(
            hid_c.T.reshape(KC, 128, BL).transpose(1, 0, 2)
        ).reshape(128, KC * BL)
        misc_c = np.concatenate([hidT_c, vt_np, bc_np], axis=1)  # [128, MISC]
        in_maps.append({
            "enc_t": enc_p,
            "uwT": uwT_np,
            "wwT": wwT_np,
            "misc": np.ascontiguousarray(misc_c),
        })
    return in_maps, Ws, rows, idx_all, counts


def _run(in_maps, Ws, trace=False):
    from concourse import bass_utils
    if Ws not in _CACHE:
        _CACHE[Ws] = _build_nc(Ws)
    nc = _CACHE[Ws]
    return bass_utils.run_bass_kernel_spmd(
        nc, in_maps, core_ids=list(range(NCORES)), trace=trace
    )


def kernel(hidden, encoder_outputs, mask, W_w, W_b, U_w, U_b, v_w,
           _trace=False, _return_bkr=False):
    hidden = np.asarray(hidden, dtype=np.float32)
    encoder_outputs = np.asarray(encoder_outputs, dtype=np.float32)
    mask = np.asarray(mask).astype(bool)
    W_w = np.asarray(W_w, dtype=np.float32)
    W_b = np.asarray(W_b, dtype=np.float32)
    U_w = np.asarray(U_w, dtype=np.float32)
    U_b = np.asarray(U_b, dtype=np.float32)
    v_w = np.asarray(v_w, dtype=np.float32)

    in_maps, Ws, rows, idx_all, counts = _prep_inputs(
        hidden, encoder_outputs, mask, W_w, W_b, U_w, U_b, v_w)
    bkr = _run(in_maps, Ws, trace=_trace)

    offs = np.concatenate([[0], np.cumsum(Ws)]).astype(int)
    out = np.zeros((B, S), np.float32)
    for c in range(NCORES):
        dev = bkr.results[c]["out"].reshape(-1)
        for b in range(BL):
            i = rows[b, c]
            cnt = counts[i]
            if cnt:
                e = dev[offs[b]:offs[b] + cnt]
                out[i, idx_all[i]] = e / e.sum()
            else:
                # fully-masked row: softmax over all -1e10 is uniform
                out[i, :] = np.float32(1.0 / S)
    if _return_bkr:
        return out, bkr
    return out


# revision 15
# speedup vs baseline: 1.6181x; 1.6181x over previous
"""Bahdanau-attention kernel for Trainium2 (8 NeuronCores, Bass/Tile).

Computation (reference, fp32):
    Wh  = hidden @ W_w.T + W_b                      # [B, H]
    Ue  = einsum('bse,he->bsh', enc^T, U_w) + U_b   # [B, S, H]
    en  = tanh(Wh[:,None,:] + Ue) @ v_w[0]          # [B, S]
    out = softmax(where(mask, -1e10, en), axis=1)

Strategy
- Data-parallel over batch: 8 rows per core, weights replicated.
- Masked positions contribute exactly 0 to the softmax (exp(-1e10)=0 in
  fp32), so the host packs only the unmasked s-columns per row and
  scatters results back (exact, not an approximation).  Rows are sorted
  by unmasked count and rank-grouped into 8 slots (one row per core per
  slot) so each slot's padded width is the max of 8 similar counts.
- The packed columns form one flat [0,TOT) space per core, processed in
  512-wide blocks that ignore slot boundaries: the main GEMM, the
  v-weighting, the partition-reduce and the exp are all slot-blind;
  only the tanh bias (Wh row) is applied per slot segment.
- Main GEMM in fp8 (e4m3) with DoubleRow perf mode: 2 e-rows per PE
  cell, 256-deep contraction per matmul, 8 matmuls per (block,
  h-chunk).  U_w is scaled by 256 before quantization (its entries are
  subnormal in e4m3 otherwise) and the tanh ACT un-scales by 1/256.
  Per-matmul LDWEIGHTS (~213 ns, no FWL with DoubleRow) is the floor,
  so block count is minimized: ceil(TOT/512) blocks.
- Wh + W_b + U_b folds into the tanh as a per-partition ACT bias; the
  Wh chains run upfront in bf16, paced by the ww DMA stream.
- Dummy matmuls on scratch SBUF warm the PE HAM clock gate (1.2 -> 2.4
  GHz) during the DMA ramp and fill early DMA-wait gaps so the clock
  never re-throttles.
- The v-projection runs on the Vector engine; a ones-vector M=1 matmul
  per block does the partition reduce.  The LAST block instead reduces
  via direct lhsT=v-chunk matmuls, removing the vector chain from the
  tail critical path.
- The device applies exp; softmax normalization (sum+divide per row)
  happens in the host unpack loop.
"""

import numpy as np
import ml_dtypes

B, S, H, E = 64, 512, 1024, 2048
NCORES = 8
BL = B // NCORES          # rows (slots) per core
HC = H // 128             # h chunks
EC = E // 128             # e (contraction) chunks
JP = EC // 2              # DoubleRow e-chunk pairs
KC = H // 128             # k chunks for the Wh matmul
USCALE = 256.0            # fp8 pre-scale for U_w

bf16 = ml_dtypes.bfloat16
f8 = ml_dtypes.float8_e4m3

_CACHE = {}

N_WARM = 24               # dummy warm-up matmuls (N=256) during DMA ramp
F_WH = 4                  # dummy filler (N=128) between Wh chains
F_MAIN = 16               # dummy filler (N=128) before first main chain


def _blocks(TOT):
    bs = []
    c = 0
    while c < TOT:
        w = min(512, TOT - c)
        bs.append((c, c + w))
        c += w
    return bs


def _build_nc(Ws):
    """Per-core program; Ws = tuple of 8 slot widths (mult of 4, <=512),
    in processing order (descending)."""
    import concourse.mybir as mybir
    import concourse.tile as tile
    from concourse import bacc

    F32 = mybir.dt.float32
    BF = mybir.dt.bfloat16
    FP8 = mybir.dt.float8e4
    AF = mybir.ActivationFunctionType
    DR = mybir.MatmulPerfMode.DoubleRow

    Ws = list(Ws)
    TOT = sum(Ws)
    off = [0]
    for w in Ws:
        off.append(off[-1] + w)
    blocks = _blocks(TOT)
    NB = len(blocks)
    # flat offset of block k in the [p, blk, jp, i, c] enc layout
    fb = [2 * EC // 2 * 0]  # placeholder, computed below
    fb = []
    acc_cols = 0
    for (c0, c1) in blocks:
        fb.append(2 * JP * c0)
        acc_cols = c1
    assert acc_cols == TOT

    # per-block slot segments: (colr0, colr1, slot) relative to block
    segs = []
    for k, (c0, c1) in enumerate(blocks):
        ss = []
        for b in range(BL):
            lo = max(c0, off[b])
            hi = min(c1, off[b + 1])
            if lo < hi:
                ss.append((lo - c0, hi - c0, b))
        segs.append(ss)

    nc = bacc.Bacc(num_swdge_queues=4)
    # enc packed fp8: [p, blk, jp, i(2), c]  flattened on dim 1
    enc_t = nc.declare_dram_parameter("enc8", [128, 2 * JP * TOT], FP8,
                                      isOutput=False)
    # U_w.T * 256 in fp8: [p(=e%128), hc, jp, i(2), v(=h%128)]
    uwT = nc.declare_dram_parameter("uw8", [128, HC * JP * 2 * 128], FP8,
                                    isOutput=False)
    # W_w.T re-chunked bf16: [p(=k%128), hc, kc, v(=h%128)]
    wwT = nc.declare_dram_parameter("wwT", [128, HC * KC * 128], BF,
                                    isOutput=False)
    # misc = hidT [.,0:KC*BL] | vt [.,KC*BL:+HC] | bc [.,KC*BL+HC:+HC]
    MISC = KC * BL + 2 * HC
    misc = nc.declare_dram_parameter("misc", [128, MISC], BF, isOutput=False)
    out_d = nc.declare_dram_parameter("out", [1, TOT], F32, isOutput=True)

    with tile.TileContext(nc) as tc:
        with (
            tc.tile_pool(name="const", bufs=1) as cst,
            tc.tile_pool(name="wpool", bufs=1) as wp,
            tc.tile_pool(name="thp", bufs=6) as thp,
            tc.tile_pool(name="accp", bufs=3) as accp,
            tc.tile_pool(name="pup", bufs=4, space="PSUM") as pup,
            tc.tile_pool(name="pep", bufs=2, space="PSUM") as pep,
            tc.tile_pool(name="pwp", bufs=1, space="PSUM") as pwp,
            tc.tile_pool(name="pwarm", bufs=1, space="PSUM") as pwarm,
        ):
            # ---- PE warm-up on scratch data (HAM clock-gate release) ---
            warm_sb = cst.tile([128, 384], BF, tag="warm")
            nc.vector.memset(warm_sb[:], 1.0)
            warm_ps = pwarm.tile([128, 256], F32, tag="wps")
            for _ in range(N_WARM):
                nc.tensor.matmul(
                    warm_ps[:], lhsT=warm_sb[:, 0:128], rhs=warm_sb[:, 128:384],
                    start=True, stop=True,
                )

            def filler(n):
                # N=128 dummy matmuls: keep the PE (and its HAM activity
                # window) busy while waiting for DMA, in fine increments
                for _ in range(n):
                    nc.tensor.matmul(
                        warm_ps[:, 0:128], lhsT=warm_sb[:, 0:128],
                        rhs=warm_sb[:, 128:256], start=True, stop=True,
                    )

            # ---- tiles ---------------------------------------------------
            misc_sb = cst.tile([128, MISC], BF, tag="misc")
            VT0 = KC * BL          # vt col offset in misc
            BC0 = KC * BL + HC     # bc col offset in misc
            ww_sb = wp.tile([128, HC * KC * 128], BF, tag="ww")
            uw_sb = wp.tile([128, HC * JP * 2 * 128], FP8, tag="uw8")
            enc_sb = wp.tile([128, 2 * JP * TOT], FP8, tag="enc8")

            WWC = KC * 128   # ww cols per hc
            UWC = JP * 2 * 128  # uw8 cols per hc

            def ww_sl(hc):
                return slice(hc * WWC, (hc + 1) * WWC)

            # q1 = sync, q2 = gpsimd.  Order = PE need order: misc+ww
            # first (Wh prologue), then uw8[0] + block-0 enc strips, then
            # the rest of uw8 interleaved with the remaining enc blocks.
            nc.gpsimd.dma_start(misc_sb[:], misc[:])
            for hc in range(0, HC, 2):
                nc.sync.dma_start(ww_sb[:, ww_sl(hc)], wwT[:, ww_sl(hc)])
            for hc in range(1, HC, 2):
                nc.gpsimd.dma_start(ww_sb[:, ww_sl(hc)], wwT[:, ww_sl(hc)])

            def uw_dma(hc):
                a = hc * UWC
                eng = (nc.sync, nc.gpsimd)[hc % 2]
                eng.dma_start(uw_sb[:, a:a + UWC], uwT[:, a:a + UWC])

            def enc_dma_quarters(k):
                c0, c1 = blocks[k]
                bw = c1 - c0
                for q in range(4):
                    a = fb[k] + q * 2 * JP * bw // 4
                    bnd = fb[k] + (q + 1) * 2 * JP * bw // 4
                    eng = (nc.sync, nc.gpsimd)[q % 2]
                    eng.dma_start(enc_sb[:, a:bnd], enc_t[:, a:bnd])

            def enc_dma_halves(k):
                c0, c1 = blocks[k]
                bw = c1 - c0
                m = fb[k] + JP * bw  # half the jp strips
                bnd = fb[k] + 2 * JP * bw
                nc.sync.dma_start(enc_sb[:, fb[k]:m], enc_t[:, fb[k]:m])
                nc.gpsimd.dma_start(enc_sb[:, m:bnd], enc_t[:, m:bnd])

            uw_dma(0)
            enc_dma_quarters(0)
            uw_dma(1)
            uw_dma(2)
            uw_dma(3)
            enc_dma_halves(1)
            uw_dma(4)
            uw_dma(5)
            enc_dma_halves(2)
            uw_dma(6)
            uw_dma(7)
            for k in range(3, NB):
                enc_dma_halves(k)

            bias_sb = cst.tile([128, HC * BL], F32, tag="bias")
            res_sb = cst.tile([1, TOT], F32, tag="res")
            ones_sb = cst.tile([128, 1], BF, tag="ones")
            nc.vector.memset(ones_sb[:], 1.0)

            # ---- Wh prologue: all 8 chains upfront ----------------------
            for hc in range(HC):
                pw = pwp.tile([128, BL], F32, tag="pw")
                for kc in range(KC):
                    nc.tensor.matmul(
                        pw[:],
                        lhsT=ww_sb[:, hc * WWC + kc * 128:
                                   hc * WWC + (kc + 1) * 128],
                        rhs=misc_sb[:, kc * BL:(kc + 1) * BL],
                        start=(kc == 0),
                        stop=(kc == KC - 1),
                    )
                nc.vector.tensor_tensor(
                    bias_sb[:, hc * BL:(hc + 1) * BL], pw[:],
                    misc_sb[:, BC0 + hc:BC0 + hc + 1].to_broadcast([128, BL]),
                    mybir.AluOpType.add,
                )
                if hc < HC - 1:
                    filler(F_WH)

            filler(F_MAIN)

            # ---- main loop over column blocks ---------------------------
            for k in range(NB):
                c0, c1 = blocks[k]
                bw = c1 - c0
                lastb = k == NB - 1
                pe_ = pep.tile([1, bw], F32, tag="pe")
                if not lastb:
                    acc = accp.tile([128, bw], F32, tag="acc")
                for hc in range(HC):
                    pu = pup.tile([128, bw], F32, tag="pu")
                    for j in range(JP):
                        rr = enc_sb[:, fb[k] + j * 2 * bw:
                                    fb[k] + (j + 1) * 2 * bw]
                        ll = uw_sb[:, hc * UWC + j * 256:
                                   hc * UWC + j * 256 + 256]
                        nc.tensor.matmul(
                            pu[:],
                            lhsT=ll.rearrange("p (i v) -> p i v", i=2),
                            rhs=rr.rearrange("p (i s) -> p i s", i=2),
                            start=(j == 0),
                            stop=(j == JP - 1),
                            perf_mode=DR,
                        )
                    th = thp.tile([128, bw], BF, tag="th")
                    for (r0, r1, b) in segs[k]:
                        nc.scalar.activation(
                            th[:, r0:r1], pu[:, r0:r1], AF.Tanh,
                            bias=bias_sb[:, hc * BL + b:hc * BL + b + 1],
                            scale=1.0 / USCALE,
                        )
                    if lastb:
                        # direct v-chunk reduce on PE: shortest tail path
                        nc.tensor.matmul(
                            pe_[0:1, :],
                            lhsT=misc_sb[:, VT0 + hc:VT0 + hc + 1],
                            rhs=th[:],
                            start=(hc == 0), stop=(hc == HC - 1),
                        )
                    else:
                        # v-weighting on the (otherwise idle) Vector engine
                        vcol = misc_sb[:, VT0 + hc:VT0 + hc + 1] \
                            .to_broadcast([128, bw])
                        if hc == 0:
                            nc.vector.tensor_tensor(
                                acc[:], th[:], vcol, mybir.AluOpType.mult)
                        else:
                            tmp = thp.tile([128, bw], F32, tag="tmp")
                            nc.vector.tensor_tensor(
                                tmp[:], th[:], vcol, mybir.AluOpType.mult)
                            nc.vector.tensor_add(acc[:], acc[:], tmp[:])
                if not lastb:
                    # single partition-reduce matmul replaces the 8 v-dots
                    accb = thp.tile([128, bw], BF, tag="accb")
                    nc.vector.tensor_copy(accb[:], acc[:])
                    nc.tensor.matmul(
                        pe_[0:1, :], lhsT=ones_sb[:, 0:1], rhs=accb[:],
                        start=True, stop=True,
                    )

                # ---- exp over the block; normalization on host ----------
                nc.scalar.activation(res_sb[0:1, c0:c1], pe_[0:1, :], AF.Exp)
                nc.sync.dma_start(out_d[0:1, c0:c1], res_sb[0:1, c0:c1])

    nc.finalize()
    return nc


def _prep_inputs(hidden, encoder_outputs, mask, W_w, W_b, U_w, U_b, v_w):
    enc_bf = encoder_outputs.astype(bf16)          # [S, B, E]
    # U_w.T * 256 -> fp8 : [p(=e%128), hc, jp, i, v(=h%128)]
    uwT_np = (np.ascontiguousarray(U_w.T) * USCALE).astype(f8)   # [E, H]
    uwT_np = np.ascontiguousarray(
        uwT_np.reshape(JP, 2, 128, HC, 128).transpose(2, 3, 0, 1, 4)
    ).reshape(128, HC * JP * 2 * 128)
    wwT_np = np.ascontiguousarray(W_w.T).astype(bf16)
    wwT_np = np.ascontiguousarray(
        wwT_np.reshape(KC, 128, HC, 128).transpose(1, 2, 0, 3)
    ).reshape(128, HC * KC * 128)
    vt_np = np.ascontiguousarray(v_w[0].reshape(HC, 128).T).astype(bf16)
    bc_np = np.ascontiguousarray((W_b + U_b).reshape(HC, 128).T).astype(bf16)

    idx_all = [np.nonzero(~mask[i])[0] for i in range(B)]
    counts = np.array([len(ix) for ix in idx_all])

    # sorted-slot packing: rank-group rows into 8 slots of 8 (one per core)
    order = np.argsort(-counts, kind="stable")
    rows = order.reshape(BL, NCORES)       # rows[b][c] = global row index
    Ws = tuple(int(max(4, 4 * -(-counts[rows[b]].max() // 4)))
               for b in range(BL))
    TOT = sum(Ws)
    off = np.concatenate([[0], np.cumsum(Ws)]).astype(int)
    blocks = _blocks(TOT)

    in_maps = []
    for c in range(NCORES):
        crows = rows[:, c]                                       # slot -> row
        # enc for this core's rows: [E, BL, S]
        enc_c = np.ascontiguousarray(enc_bf[:, crows, :].transpose(2, 1, 0))
        enc_flat = np.zeros((E, TOT), np.float32)
        for b in range(BL):
            ix = idx_all[crows[b]]
            cnt = len(ix)
            if cnt:
                enc_flat[:, off[b]:off[b] + cnt] = enc_c[:, b, ix]
        enc8 = enc_flat.astype(f8)                               # [E, TOT]
        # [E, TOT] -> [jp, i, p, col] -> [p, jp, i, col]
        enc8 = enc8.reshape(JP, 2, 128, TOT).transpose(2, 0, 1, 3)
        parts = [np.ascontiguousarray(enc8[:, :, :, c0:c1]).reshape(128, -1)
                 for (c0, c1) in blocks]
        enc_p = np.ascontiguousarray(np.concatenate(parts, axis=1))
        hid_c = hidden[crows].astype(bf16)                       # [BL, H]
        hidT_c = np.ascontiguousarray(
            hid_c.T.reshape(KC, 128, BL).transpose(1, 0, 2)
        ).reshape(128, KC * BL)
        misc_c = np.concatenate([hidT_c, vt_np, bc_np], axis=1)  # [128, MISC]
        in_maps.append({
            "enc8": enc_p,
            "uw8": uwT_np,
            "wwT": wwT_np,
            "misc": np.ascontiguousarray(misc_c),
        })
    return in_maps, Ws, rows, idx_all, counts


def _run(in_maps, Ws, trace=False):
    from concourse import bass_utils
    if Ws not in _CACHE:
        _CACHE[Ws] = _build_nc(Ws)
    nc = _CACHE[Ws]
    return bass_utils.run_bass_kernel_spmd(
        nc, in_maps, core_ids=list(range(NCORES)), trace=trace
    )


def kernel(hidden, encoder_outputs, mask, W_w, W_b, U_w, U_b, v_w,
           _trace=False, _return_bkr=False):
    hidden = np.asarray(hidden, dtype=np.float32)
    encoder_outputs = np.asarray(encoder_outputs, dtype=np.float32)
    mask = np.asarray(mask).astype(bool)
    W_w = np.asarray(W_w, dtype=np.float32)
    W_b = np.asarray(W_b, dtype=np.float32)
    U_w = np.asarray(U_w, dtype=np.float32)
    U_b = np.asarray(U_b, dtype=np.float32)
    v_w = np.asarray(v_w, dtype=np.float32)

    in_maps, Ws, rows, idx_all, counts = _prep_inputs(
        hidden, encoder_outputs, mask, W_w, W_b, U_w, U_b, v_w)
    bkr = _run(in_maps, Ws, trace=_trace)

    offs = np.concatenate([[0], np.cumsum(Ws)]).astype(int)
    out = np.zeros((B, S), np.float32)
    for c in range(NCORES):
        dev = bkr.results[c]["out"].reshape(-1)
        for b in range(BL):
            i = rows[b, c]
            cnt = counts[i]
            if cnt:
                e = dev[offs[b]:offs[b] + cnt]
                out[i, idx_all[i]] = e / e.sum()
            else:
                # fully-masked row: softmax over all -1e10 is uniform
                out[i, :] = np.float32(1.0 / S)
    if _return_bkr:
        return out, bkr
    return out
